# revision 87
# baseline (speedup 1.0000x reference)
"""Trainium2 Bass kernel for nn_DeformableAttention1D (PE-centric redesign).

Shapes (hardcoded): B=4, L=4096, dm=512, H=8 heads, D=64, P=4 points.
Sharding: 8 cores; core c handles batch b=c//2, query half c%2 (2048 queries).

Design: token-major tiles of TL=122 tokens with a WIN=128 position window
(window start = tile_start - 3).  The sampling offsets satisfy |off| < 3, so
token l only ever touches positions l-3..l+3, all inside its tile's window.

  scores   S[l,h,m]  = q[l,h,:].k[m,h,:]          PE matmul per (tile, head)
  band     Sb[l,h,j] = S[l,h, l_loc+j]            DMA shear via DRAM staging
                                                  (j = dlt+3, dlt in [-3,4))
  hat      hv[l,h,p,j] = max(1-|r_p - (j-3)|, 0)  DVE (lerp hat weights)
  dot      dot = sum_j hv*Sb; w = softmax_p(dot+logits)        DVE small ops
  combine  C[l,h,j] = sum_p w_p*hv[...,j]         DVE
  A^T      A[m,h,l_loc] = C[l=m-(j-3)][...,j]     7 shifted DMAs + gpsimd
                                                  local_scatter (per-partition
                                                  static indices, zero-fills)
  output   out[hd,l] = sum_m v[m,hd]*A[m,h,l]     PE matmul per (tile, head)
                                                  (odd heads via tile_position
                                                  column offset 64)

The whole thing is pipelined in 3 tile-groups: each group's projections,
scores, softmax, combine, and output projection are emitted together so the
engines overlap across groups.  All input layout work (transposes, fp16
casts, weight chunking, scatter indices) happens on the host in numpy.
"""

import numpy as np

H, PP = 8, 4            # heads, sample points
DM = 512
L = 4096
B = 4
NCORES = 8
LQ = 2048               # tokens per core
TL = 122                # tokens per tile
NT = 17                 # tiles (17*122 = 2074 >= 2048)
WIN = 128               # window positions per tile, starts at tile*TL - 3
NJ = 8                  # band slots j (dlt = j-3; slot 7 has zero hat weight)
QW = 2080               # padded q width
KW = 2088               # padded kv width: local tokens [-8, 2080)
KOFF = 8                # kv column offset of local token 0
STAGEN = 131136         # DRAM shear staging: 128*1025 + 896 + 8, rounded up
GROUPS = [(0, 4), (4, 7), (7, 10), (10, 13), (13, 16), (16, 17)]

_CACHE = {}


def _chunks(a, b, maxn):
    k = -(-(b - a) // maxn)
    n = -(-(b - a) // k)
    return [(x, min(x + n, b)) for x in range(a, b, n)]


def _build_program():
    import contextlib

    import concourse.mybir as mybir
    import concourse.tile as tile
    from concourse.ap import AP
    from concourse import bacc

    f32, f16 = mybir.dt.float32, mybir.dt.float16
    i16 = mybir.dt.int16
    A = mybir.AluOpType
    AF = mybir.ActivationFunctionType
    X = mybir.AxisListType.X

    nc = bacc.Bacc(
        "TRN2",
        target_bir_lowering=False,
        debug=False,
        enable_asserts=False,
        num_devices=NCORES,
    )

    # packed weights: wqs(2048) wks(2048) | wvr(2048) wor(2048) wolr(256)
    WPACK = 2048 * 4 + 256
    # packed smalls: lo(17) hi(17) jm3(8) jm1(8)
    SPACK = 50
    qT_d = nc.dram_tensor("qT", [128, 4, QW], f16, kind="ExternalInput")
    kvT_d = nc.dram_tensor("kvT", [128, 4, KW], f16, kind="ExternalInput")
    wp_d = nc.dram_tensor("wpack", [128, WPACK], f16, kind="ExternalInput")
    sp_d = nc.dram_tensor("spack", [128, SPACK], f16, kind="ExternalInput")
    sci_d = nc.dram_tensor("scidx", [128, 7 * (H // 2)], i16, kind="ExternalInput")
    y_d = nc.dram_tensor("y16", [LQ, DM], f16, kind="ExternalOutput")

    with tile.TileContext(nc) as tc:
        with contextlib.ExitStack() as ctx:
            const = ctx.enter_context(tc.tile_pool(name="const", bufs=1))
            big = ctx.enter_context(tc.tile_pool(name="big", bufs=1))
            work = ctx.enter_context(tc.tile_pool(name="work", bufs=1))
            psum = ctx.enter_context(tc.tile_pool(name="psum", bufs=2, space="PSUM"))
            dram = ctx.enter_context(tc.tile_pool(name="dram", bufs=1, space="DRAM"))

            # ---- constants + activations; q/k weights first so the
            # projections can start as soon as qT/kvT chunks land
            wp = const.tile([128, WPACK], f16, name="wp")
            sp = const.tile([128, SPACK], f16, name="sp")
            sci = const.tile([128, 7 * (H // 2)], i16, name="sci")
            qT = big.tile([128, 4, QW], f16, name="qT")
            kvT = big.tile([128, 4, KW], f16, name="kvT")
            nc.sync.dma_start(wp[:, 0:2048], wp_d[:, 0:2048])
            for ci in range(4):
                nc.sync.dma_start(qT[:, ci], qT_d[:, ci])
            nc.sync.dma_start(wp[:, 2048:4096], wp_d[:, 2048:4096])
            for ci in range(4):
                nc.sync.dma_start(kvT[:, ci], kvT_d[:, ci])
            nc.sync.dma_start(wp[:, 4096:WPACK], wp_d[:, 4096:WPACK])
            nc.sync.dma_start(sp[:], sp_d[:])
            nc.sync.dma_start(sci[:], sci_d[:])
            wqs = wp[:, 0:2048].rearrange("p (ci co m) -> p ci co m", ci=4, co=4)
            wks = wp[:, 2048:4096].rearrange("p (ci co m) -> p ci co m", ci=4, co=4)
            wvr = wp[:, 4096:6144].rearrange("p (ci n) -> p ci n", ci=4)
            wor = wp[:, 6144:8192].rearrange("p (ci n) -> p ci n", ci=4)
            wolr = wp[:, 8192:8448].rearrange("p (ci n) -> p ci n", ci=4)
            lo_sb = sp[:, 0:NT]
            hi_sb = sp[:, NT:2 * NT]
            jm3 = sp[:, 34:42]
            jm1 = sp[:, 42:50]

            # ---- persistent mid tensors
            qT2 = big.tile([128, 4, QW], f16, name="qT2")
            kT2 = big.tile([128, 4, KW], f16, name="kT2")
            qT2b = big.tile([64, 4, QW], f16, name="qT2b")
            kT2b = big.tile([64, 4, KW], f16, name="kT2b")
            v_win = big.tile([128, NT, DM], f16, name="v_win")
            ol = big.tile([128, NT, 64], f16, name="ol")
            Sband = big.tile([128, NT, H, NJ], f16, name="Sband")
            C2 = big.tile([128, NJ, NT, H], f16, name="C2")
            Cst = big.tile([128, 7, NT, H], f16, name="Cst")
            CsA = big.tile([128, NT, 2, 7, H // 2], f16, name="CsA")
            outT = big.tile([128, 4, QW], f16, name="outT")
            nc.gpsimd.memset(ol[96:128], 0.0)
            nc.gpsimd.memset(Cst[:], 0.0)

            stage = dram.tile([2, STAGEN], f16, name="stage")
            zpad = work.tile([1, 64], f16, name="zpad")
            nc.gpsimd.memset(zpad[:], 0.0)
            NSB = 128 * 1024
            for tb in range(2):
                nc.sync.dma_start(
                    stage[tb, NSB:STAGEN].rearrange("(o n) -> o n", o=1), zpad[:])

            # group-sized DVE scratch (allocated once at max group size)
            MXG = max(b - a for a, b in GROUPS)
            MX = MXG * H * PP
            r_f = work.tile([128, MXG, H * PP], f16, name="r_f")
            R2_f = work.tile([128, MX, NJ], f16, name="R2_f")
            hv_f = work.tile([128, MXG, H * PP, NJ], f16, name="hv_f")
            w2_f = work.tile([128, MX, NJ], f16, name="w2_f")
            pd_f = work.tile([128, MXG, H, PP, NJ], f16, name="pd_f")
            d4_f = work.tile([128, MXG, H, PP, 4], f16, name="d4_f")
            d2_f = work.tile([128, MXG, H, PP, 2], f16, name="d2_f")
            z_f = work.tile([128, MXG, H, PP], f16, name="z_f")
            e_f = work.tile([128, MXG, H, PP], f16, name="e_f")
            ssum_f = work.tile([128, MXG, H], f32, name="ssum_f")
            rec_f = work.tile([128, MXG, H], f32, name="rec_f")
            wts_f = work.tile([128, MXG, H, PP], f16, name="wts_f")
            c2_f = work.tile([128, MXG, H, 2, NJ], f16, name="c2_f")

            # two hat buffers so group g+1's hat build can overlap
            # group g's softmax/combine reads
            hv_a = work.tile([128, MXG, H * PP, NJ], f16, name="hv_a")
            hv_bufs = [hv_f, hv_a]

            def emit_proj(gi):
                g0, g1 = GROUPS[gi]
                L0, L1 = g0 * TL, min(g1 * TL + 6, QW)
                K0, K1 = g0 * TL, min(g1 * TL + 16, KW)
                (a, b), = _chunks(L0, L1, 512)
                pqs = [psum.tile([128, 512], f32, name=f"pq{co}", tag="pj", bufs=4)
                       for co in range(4)]
                for ci in range(4):
                    for co in range(4):
                        nc.tensor.matmul(pqs[co][:, 0:b - a], wqs[:, ci, co],
                                         qT[:, ci, a:b],
                                         start=(ci == 0), stop=(ci == 3))
                for co in range(4):
                    if co < 2:
                        nc.scalar.activation(qT2[:, co, a:b],
                                             pqs[co][:, 0:b - a], AF.Copy)
                    else:
                        nc.vector.tensor_copy(out=qT2[:, co, a:b],
                                              in_=pqs[co][:, 0:b - a])
                (a, b), = _chunks(K0, K1, 512)
                pks = [psum.tile([128, 512], f32, name=f"pk{co}", tag="pj", bufs=4)
                       for co in range(4)]
                for ci in range(4):
                    for co in range(4):
                        nc.tensor.matmul(pks[co][:, 0:b - a], wks[:, ci, co],
                                         kvT[:, ci, a:b],
                                         start=(ci == 0), stop=(ci == 3))
                for co in range(4):
                    if co < 2:
                        nc.scalar.activation(kT2[:, co, a:b],
                                             pks[co][:, 0:b - a], AF.Copy)
                    else:
                        nc.vector.tensor_copy(out=kT2[:, co, a:b],
                                              in_=pks[co][:, 0:b - a])
                for t in range(g0, g1):
                    w0 = t * TL + KOFF - 3
                    pv = psum.tile([128, 512], f32, name="pv", tag="pj", bufs=4)
                    for ci in range(4):
                        nc.tensor.matmul(pv[:], kvT[:, ci, w0:w0 + WIN],
                                         wvr[:, ci],
                                         start=(ci == 0), stop=(ci == 3))
                    nc.scalar.activation(v_win[:, t], pv[:], AF.Copy)
                for t in range(g0, g1):
                    po = psum.tile([128, 512], f32, name="po", tag="pj", bufs=4)
                    for ci in range(4):
                        nc.tensor.matmul(po[0:TL, 0:64],
                                         qT[:, ci, t * TL:t * TL + TL],
                                         wolr[:, ci],
                                         start=(ci == 0), stop=(ci == 3))
                    nc.vector.tensor_copy(out=ol[0:TL, t], in_=po[0:TL, 0:64])
                # odd heads live at partitions 64..128 of their chunk; mixing
                # row-group-64 stationaries with row-group-0 ones in a single
                # program crashes the runtime, so shift them down via DMA
                # (on the Pool/SWDGE queue, ahead of this group's stage writes)
                nc.gpsimd.dma_start(qT2b[:, :, L0:L1], qT2[64:128, :, L0:L1])
                nc.gpsimd.dma_start(kT2b[:, :, K0:K1], kT2[64:128, :, K0:K1])

            def emit_hat(gi):
                g0, g1 = GROUPS[gi]
                NG = g1 - g0
                ts = slice(g0, g1)
                MG = NG * H * PP
                r = r_f[:, 0:NG]
                nc.vector.tensor_tensor(
                    out=r[:], in0=ol[:, ts, 0:H * PP],
                    in1=lo_sb[:, ts].unsqueeze(2).broadcast_to(
                        (128, NG, H * PP)), op=A.max)
                nc.vector.tensor_tensor(
                    out=r[:], in0=r[:],
                    in1=hi_sb[:, ts].unsqueeze(2).broadcast_to(
                        (128, NG, H * PP)), op=A.min)
                R2 = R2_f[:, 0:MG]
                nc.vector.tensor_scalar(
                    out=R2[:],
                    in0=(r[:].rearrange("p t m -> p (t m)").unsqueeze(2)
                         .broadcast_to((128, MG, NJ))),
                    scalar1=1.0, scalar2=None, op0=A.add)
                hv = hv_bufs[gi % 2][:, 0:NG]
                w2 = w2_f[:, 0:MG]
                nc.vector.tensor_tensor(
                    out=hv[:].rearrange("p t m j -> p (t m) j"), in0=R2[:],
                    in1=jm3.unsqueeze(1).broadcast_to((128, MG, NJ)),
                    op=A.subtract)
                nc.vector.tensor_tensor(
                    out=w2[:],
                    in0=jm1.unsqueeze(1).broadcast_to((128, MG, NJ)),
                    in1=R2[:], op=A.subtract)
                nc.vector.tensor_tensor(
                    out=hv[:].rearrange("p t m j -> p (t m) j"),
                    in0=hv[:].rearrange("p t m j -> p (t m) j"), in1=w2[:],
                    op=A.min)
                nc.vector.tensor_scalar(
                    out=hv[:], in0=hv[:], scalar1=0.0, scalar2=None, op0=A.max)

            def emit_passA(gi):
                g0, g1 = GROUPS[gi]
                for t in range(g0, g1):
                    ssb = work.tile([128, H, WIN], f16, name="ssb", tag="ssb",
                                    bufs=2)
                    for half in range(2):
                        ph = psum.tile([128, 4, WIN], f32, name="ps",
                                       tag="ps", bufs=2)
                        for hh in range(4):
                            h = half * 4 + hh
                            co = h // 2
                            qa = qT2 if h % 2 == 0 else qT2b
                            ka = kT2 if h % 2 == 0 else kT2b
                            nc.tensor.matmul(
                                ph[:, hh, :],
                                qa[0:64, co, t * TL:t * TL + 128],
                                ka[0:64, co, t * TL + 5:t * TL + 5 + WIN],
                                start=True, stop=True)
                        if half == 0:
                            nc.scalar.activation(ssb[:, 0:4], ph[:], AF.Copy)
                        else:
                            nc.vector.tensor_copy(out=ssb[:, 4:8], in_=ph[:])
                    tb = t % 2
                    nc.sync.dma_start(
                        stage[tb, 0:NSB].rearrange("(p n) -> p n", p=128),
                        ssb[:].rearrange("p h m -> p (h m)"))
                    shear = AP(stage[:].tensor, tb * STAGEN,
                               [[H * WIN + 1, 128], [WIN, H], [1, NJ]])
                    nc.sync.dma_start(Sband[:, t], shear)

            def emit_phase3(gi):
                g0, g1 = GROUPS[gi]
                NG = g1 - g0
                ts = slice(g0, g1)
                hv5 = hv_bufs[gi % 2][:, 0:NG].rearrange(
                    "p t (h q) j -> p t h q j", h=H)
                pd = pd_f[:, 0:NG]
                d4 = d4_f[:, 0:NG]
                d2 = d2_f[:, 0:NG]
                z = z_f[:, 0:NG]
                e = e_f[:, 0:NG]
                ssum = ssum_f[:, 0:NG]
                rec = rec_f[:, 0:NG]
                wts = wts_f[:, 0:NG]
                c2 = c2_f[:, 0:NG]
                for q in range(PP):
                    nc.vector.tensor_tensor(
                        out=pd[:, :, :, q], in0=hv5[:, :, :, q],
                        in1=Sband[:, ts], op=A.mult)
                for q in range(PP):
                    nc.vector.tensor_tensor(
                        out=d4[:, :, :, q], in0=pd[:, :, :, q, 0:4],
                        in1=pd[:, :, :, q, 4:8], op=A.add)
                for q in range(PP):
                    nc.vector.tensor_tensor(
                        out=d2[:, :, :, q], in0=d4[:, :, :, q, 0:2],
                        in1=d4[:, :, :, q, 2:4], op=A.add)
                nc.vector.tensor_tensor(
                    out=z[:].unsqueeze(4), in0=d2[:, :, :, :, 0:1],
                    in1=d2[:, :, :, :, 1:2], op=A.add)
                nc.vector.tensor_tensor(
                    out=z[:].rearrange("p t h q -> p t (h q)"),
                    in0=z[:].rearrange("p t h q -> p t (h q)"),
                    in1=ol[:, ts, H * PP:2 * H * PP], op=A.add)
                nc.scalar.activation(
                    e[:].rearrange("p t h q -> p (t h q)"),
                    z[:].rearrange("p t h q -> p (t h q)"), AF.Exp)
                nc.vector.tensor_reduce(ssum[:], e[:], axis=X, op=A.add)
                nc.vector.reciprocal(rec[:], ssum[:])
                nc.vector.tensor_tensor(
                    out=wts[:], in0=e[:],
                    in1=rec[:].unsqueeze(3).broadcast_to((128, NG, H, PP)),
                    op=A.mult)
                for q in range(PP):
                    nc.vector.tensor_tensor(
                        out=pd[:, :, :, q], in0=hv5[:, :, :, q],
                        in1=wts[:, :, :, q].unsqueeze(3).broadcast_to(
                            (128, NG, H, NJ)),
                        op=A.mult)
                nc.vector.tensor_tensor(
                    out=c2[:], in0=pd[:, :, :, 0:2], in1=pd[:, :, :, 2:4],
                    op=A.add)
                nc.vector.tensor_tensor(
                    out=C2[:, :, ts].transpose([0, 2, 3, 1]).unsqueeze(3),
                    in0=c2[:, :, :, 0:1], in1=c2[:, :, :, 1:2], op=A.add)
                for j in range(7):
                    nc.sync.dma_start(Cst[j:128, j, ts], C2[0:128 - j, j, ts])
                for half in range(2):
                    nc.vector.tensor_copy(
                        out=CsA[:, ts, half],
                        in_=Cst[:, :, ts, half * 4:half * 4 + 4]
                        .transpose([0, 2, 1, 3]))

            def emit_passB(gi, ylim=0):
                g0, g1 = GROUPS[gi]
                for t in range(g0, g1):
                    at = work.tile([128, 2, H // 2, TL], f16, name="at",
                                   tag="at", bufs=4)
                    for half in range(2):
                        nc.gpsimd.local_scatter(
                            at[:, half].rearrange("p h l -> p (h l)"),
                            CsA[:, t, half].rearrange("p j h -> p (j h)"),
                            sci[:], channels=128, num_elems=(H // 2) * TL,
                            num_idxs=7 * (H // 2))
                    pv2 = psum.tile([128, 4, TL], f32, name="pv2", tag="pv",
                                    bufs=1)
                    for h in range(H):
                        po2 = (h % 2) * 64
                        nc.tensor.matmul(
                            pv2[po2:po2 + 64, h // 2, :],
                            v_win[:, t, h * 64:(h + 1) * 64],
                            at[:, h // 4, h % 4], start=True, stop=True,
                            tile_position=(0, po2), skip_group_check=True)
                    nc.scalar.activation(
                        outT[:, :, t * TL:(t + 1) * TL], pv2[:], AF.Copy)
                    emit_y(ylim, max_tiles=1)


            def emit_y(tok_lim, max_tiles=99):
                while ((emit_y.done + 1) * 128 <= tok_lim
                       and emit_y.done < 16 and max_tiles > 0):
                    max_tiles -= 1
                    tt = emit_y.done
                    py = psum.tile([128, 512], f32, name="py", tag="py",
                                   bufs=1)
                    for ki in range(4):
                        nc.tensor.matmul(
                            py[:], outT[:, ki, tt * 128:(tt + 1) * 128],
                            wor[:, ki], start=(ki == 0), stop=(ki == 3))
                    ysb = work.tile([128, DM], f16, name="ysb", tag="ysb",
                                    bufs=2)
                    if tt % 2 == 0:
                        nc.scalar.activation(ysb[:], py[:], AF.Copy)
                    else:
                        nc.vector.tensor_copy(out=ysb[:], in_=py[:])
                    nc.sync.dma_start(y_d[tt * 128:(tt + 1) * 128], ysb[:])
                    emit_y.done += 1

            emit_y.done = 0
            emit_proj(0)
            emit_hat(0)
            for gi in range(len(GROUPS)):
                emit_passA(gi)
                if gi + 1 < len(GROUPS):
                    emit_proj(gi + 1)
                    emit_hat(gi + 1)
                emit_phase3(gi)
                # y lags one group, interleaved between v-combine tiles so
                # the PE has work while Pool runs the scatters
                ylim = GROUPS[gi - 1][1] * TL if gi >= 1 else 0
                emit_passB(gi, ylim)
                if gi >= 1:
                    emit_y(GROUPS[gi - 1][1] * TL)
            emit_y(LQ)

    nc.compile()
    return nc


def _host_prep(inputs):
    """Per-core input maps (all layout work in numpy)."""
    q_in = np.asarray(inputs["q_in"], np.float32)
    kv_in = np.asarray(inputs["kv_in"], np.float32)
    Wq = np.asarray(inputs["Wq"], np.float32)
    Wk = np.asarray(inputs["Wk"], np.float32)
    Wv = np.asarray(inputs["Wv"], np.float32)
    Woff = np.asarray(inputs["Woff"], np.float32)
    Wa = np.asarray(inputs["Wa"], np.float32)
    Wo = np.asarray(inputs["Wo"], np.float32)

    # biases are structurally zero for this problem instance; bo is added on
    # the host below, the others must be zero for the kernel to be exact.
    for nm in ("bq", "bk", "bv", "boff", "ba"):
        assert not np.any(np.asarray(inputs[nm])), f"nonzero bias {nm} unsupported"

    D = DM // H
    wqs = np.ascontiguousarray(
        (Wq.T / np.sqrt(D)).reshape(4, 128, 4, 128).transpose(1, 0, 2, 3))
    wks = np.ascontiguousarray(Wk.T.reshape(4, 128, 4, 128).transpose(1, 0, 2, 3))
    wvr = np.ascontiguousarray(Wv.T.reshape(4, 128, DM).transpose(1, 0, 2))
    wor = np.ascontiguousarray(Wo.T.reshape(4, 128, DM).transpose(1, 0, 2))
    wolT = np.concatenate([Woff.T, Wa.T], axis=1)  # [512, 64]
    wolr = np.ascontiguousarray(wolT.reshape(4, 128, 64).transpose(1, 0, 2))
    wpack = np.concatenate(
        [wqs.reshape(128, -1), wks.reshape(128, -1), wvr.reshape(128, -1),
         wor.reshape(128, -1), wolr.reshape(128, -1)], axis=1
    ).astype(np.float16)

    # static scatter indices: at[p -> (h, lw=p-j)] = CsA[p, (j, h)]
    sci = np.full((128, 7 * (H // 2)), -1, np.int16)
    for p in range(128):
        for j in range(7):
            lw = p - j
            if 0 <= lw < TL:
                for h in range(H // 2):
                    sci[p, j * (H // 2) + h] = h * TL + lw
    jm3 = (np.arange(NJ, dtype=np.float32) - 3)

    in_maps = []
    for c in range(NCORES):
        b, half = c // 2, c % 2
        l0 = half * LQ
        qs = q_in[b, l0:l0 + LQ]                       # [2048, 512]
        qT = np.zeros((128, 4, QW), np.float16)
        qT[:, :, :LQ] = np.ascontiguousarray(
            qs.T.reshape(4, 128, LQ)).transpose(1, 0, 2)
        rows = np.clip(np.arange(l0 - KOFF, l0 - KOFF + KW), 0, L - 1)
        kvs = kv_in[b, rows]                           # [KW, 512]
        kvT = np.ascontiguousarray(
            kvs.T.reshape(4, 128, KW).transpose(1, 0, 2)).astype(np.float16)

        lo = np.zeros((128, NT), np.float32)
        hi = np.zeros((128, NT), np.float32)
        for t in range(NT):
            for p in range(min(TL, 128)):
                l = t * TL + p
                if l < LQ:
                    gl = l0 + l
                    lo[p, t] = max(-float(gl), -3.0)
                    hi[p, t] = min(float(L - 1 - gl), 3.0)
        spack = np.concatenate(
            [lo, hi, np.tile(jm3[None, :], (128, 1)),
             np.tile(jm3[None, :] + 2.0, (128, 1))], axis=1).astype(np.float16)
        in_maps.append({
            "qT": qT, "kvT": kvT, "wpack": wpack, "spack": spack, "scidx": sci,
        })
    return in_maps


def kernel(**inputs):
    if "nc" not in _CACHE:
        _CACHE["nc"] = _build_program()
    nc = _CACHE["nc"]

    from concourse.bass_utils import run_bass_kernel_spmd

    in_maps = _host_prep(inputs)
    res = run_bass_kernel_spmd(nc, in_maps, core_ids=list(range(NCORES)))
    out = np.empty((B, L, DM), np.float32)
    for c in range(NCORES):
        b, half = c // 2, c % 2
        out[b, half * LQ:(half + 1) * LQ] = res.results[c]["y16"].astype(np.float32)
    out += np.asarray(inputs["bo"], np.float32)[None, None, :]
    return out
